# revision 1
# baseline (speedup 1.0000x reference)
"""Bass/Tile kernel builder for the two-stage attention block.

Layout strategy: everything on-chip is kept transposed relative to the
reference ([feature, token] with feature on partitions) so that both
attention stages chain with zero on-chip transposes:

  QT/KT  = W.T @ x.T       : matmul(lhsT=W_chunk, rhs=xT_chunk)   -> [c, i]
  V      = x @ W           : matmul(lhsT=xT_chunk, rhs=Wv_chunk)  -> [j, c]
  S^T    = (q@k.T).T       : matmul(lhsT=KT_h, rhs=QT_h)          -> [j, i]
  P^T    = act(S^T * m^T)  : elementwise (orientation-agnostic)
  O^T    = (P@v).T         : matmul(lhsT=V_h, rhs=P^T_h)          -> [d, i]
  heads stacked on partitions -> O^T == out1.T, which directly feeds the
  next projection as lhsT chunks.

Softmax (stage 2) denominator: V2 is stored head-padded [j, 8*65] with a
ones column appended per head, so the apply matmul's PSUM row 64 is
sum_j exp(s) for free. Normalization happens on the small [64, 1024]
apply output, not the big [1024, 1024] probability matrix.

All matmul operands are float32r (~1.5e-4 rel err, full PE speed at
N=512 vs 4x slowdown for fp32).
"""

from contextlib import ExitStack

import concourse.bass as bass
import concourse.tile as tile
from concourse import mybir
from concourse.vector_clock import ScopedClock

F32 = mybir.dt.float32
F32R = mybir.dt.float32r
AF = mybir.ActivationFunctionType
ALU = mybir.AluOpType

N, DIM, H, D = 1024, 512, 8, 64
SCALE = DIM**-0.5
KC = DIM // 128  # contraction chunks for projections
JC = N // 128  # key-side chunks (128 wide)
IC = N // 512  # query-side chunks (512 wide)
EXP_BIAS = -15.0
VP = D + 1  # per-head width in the padded V2 (ones column appended)


# ---------------------------------------------------------------------------
# Walrus in this container rejects instructions with >1 sync-wait.
# Split: hoist extra waits onto single-wait NoOps inserted just before.
def legalize_single_wait(nc):
    n_split = 0
    for fn in nc.m.functions:
        for blk in fn.blocks:
            insts = list(blk.instructions)
            out = []
            changed = False
            for inst in insts:
                si = inst.sync_info
                waits = list(si.on_wait) if (si is not None and si.on_wait) else []
                if len(waits) > 1:
                    changed = True
                    n_split += len(waits) - 1
                    for w in waits[:-1]:
                        nop = mybir.InstNoOp(
                            name=nc.get_next_instruction_name(),
                            sync_info=mybir.SyncInfo(on_wait=[w], on_update=[]),
                            bass_nofuse=True,
                            engine=inst.engine,
                        )
                        nc.register_instruction(nop)
                        out.append(nop)
                    si.on_wait = [waits[-1]]
                out.append(inst)
            if changed:
                blk.instructions = out
    return n_split


def _patched_drain_and_barrier(self, tick_clock, wait_clock):
    drain_inst = self.nc.sync.drain()
    wait_clock.add_sem_waits(
        drain_inst.ins, ScopedClock({None: tick_clock.global_clock})
    )
    si = drain_inst.ins.sync_info
    waits = list(si.on_wait or []) if si is not None else []
    if len(waits) > 1:
        si.on_wait = [waits[0]]
        for w in waits[1:]:
            extra = self.nc.sync.drain()
            esi = extra.ins.sync_info
            if esi is None:
                extra.ins.sync_info = mybir.SyncInfo(on_wait=[w], on_update=[])
            else:
                esi.on_wait = [w]

    self.nc.all_engine_barrier()
    assert self.sems is not None
    popped = self.nc._tile_sem_poison_stack.pop()
    assert popped is self._sem_poison
    self.nc.clear_and_free_semaphores(list(self.sems.allocated().values()))
    self.nc.all_engine_barrier()


def install_patches():
    tile.TileContext._drain_and_barrier = _patched_drain_and_barrier


# ---------------------------------------------------------------------------


def _qkv_proj_T(nc, pool_ps, w_sb, bias_sb, src_sb, dst, col0):
    """dst[c, i] (c on partitions, 2 heads per tile) for cols [col0, col0+512)
    of the weight: dst = W[:, col0:col0+512].T @ src + b."""
    for t in range(4):  # c-chunks of 128 (head pairs)
        for ic in range(IC):
            ps = pool_ps.tile([128, 512], F32, tag="proj_ps")
            for kc in range(KC):
                nc.tensor.matmul(
                    ps[:],
                    w_sb[kc][:, col0 + t * 128 : col0 + (t + 1) * 128],
                    src_sb[kc][:, ic * 512 : (ic + 1) * 512],
                    start=(kc == 0),
                    stop=(kc == KC - 1),
                )
            nc.vector.tensor_scalar_add(
                dst[t][:, ic * 512 : (ic + 1) * 512], ps[:], bias_sb[t][:]
            )


def build_body(ctx: ExitStack, tc: tile.TileContext, d, out_ap):
    nc = tc.nc

    const_pool = ctx.enter_context(tc.tile_pool(name="const", bufs=1))

    # --- persistent small constants -------------------------------------
    def load_bias_cols(name, src, off):
        """[128,1] per c-chunk bias tiles (c on partitions)."""
        tiles = []
        for t in range(4):
            b = const_pool.tile([128, 1], F32, name=f"{name}_{t}")
            nc.gpsimd.dma_start(b[:], src[off + t * 128 : off + (t + 1) * 128])
            tiles.append(b)
        return tiles

    def load_bias_bcast(name, src, off, width):
        """bias broadcast across partitions -> [128, width] f32."""
        row = const_pool.tile([1, width], F32, name=f"{name}_row")
        nc.gpsimd.dma_start(row[:], src[off : off + width])
        full = const_pool.tile([128, width], F32, name=f"{name}_full")
        nc.gpsimd.partition_broadcast(full[:], row[:])
        return full

    bq1 = load_bias_cols("bq1", d["bqkv1"], 0)
    bk1 = load_bias_cols("bk1", d["bqkv1"], DIM)
    bv1_b = load_bias_bcast("bv1", d["bqkv1"], 2 * DIM, DIM)
    bq2 = load_bias_cols("bq2", d["bqkv2"], 0)
    bk2 = load_bias_cols("bk2", d["bqkv2"], DIM)
    bv2_b = load_bias_bcast("bv2", d["bqkv2"], 2 * DIM, DIM)
    bnn_b = load_bias_bcast("bnn", d["bnn1"], 0, DIM)

    ones8 = const_pool.tile([128, H], F32R, name="ones8")
    nc.gpsimd.dma_start(ones8[:], d["ones"][:, :])

    # --- stage-1 persistent tensors --------------------------------------
    qk1_pool = ctx.enter_context(tc.tile_pool(name="qk1", bufs=1))
    QT1 = [qk1_pool.tile([128, N], F32R, name=f"QT1_{t}") for t in range(4)]
    KT1 = [qk1_pool.tile([128, N], F32R, name=f"KT1_{t}") for t in range(4)]
    V1 = [qk1_pool.tile([128, DIM], F32R, name=f"V1_{j}") for j in range(JC)]
    o1_pool = ctx.enter_context(tc.tile_pool(name="o1", bufs=1))
    O1T = [o1_pool.tile([128, N], F32R, name=f"O1T_{t}") for t in range(4)]

    mask_pool = ctx.enter_context(tc.tile_pool(name="mask", bufs=1))
    maskT = [mask_pool.tile([128, N], F32, name=f"maskT_{j}") for j in range(JC)]
    for j in range(JC):
        nc.gpsimd.dma_start(maskT[j][:], d["maskT"][j * 128 : (j + 1) * 128, :])

    # =====================================================================
    # Phase 1: stage-1 projections
    # =====================================================================
    with tc.tile_pool(name="xw1", bufs=1) as xw1_pool, \
         tc.tile_pool(name="ps1", bufs=4, space="PSUM") as ps1_pool:
        xT = [xw1_pool.tile([128, N], F32R, name=f"xT_{k}") for k in range(KC)]
        for k in range(KC):
            nc.gpsimd.dma_start(xT[k][:], d["xT"][k * 128 : (k + 1) * 128, :])
        W1 = [xw1_pool.tile([128, 3 * DIM], F32R, name=f"W1_{k}") for k in range(KC)]
        for k in range(KC):
            nc.gpsimd.dma_start(W1[k][:], d["Wqkv1"][k * 128 : (k + 1) * 128, :])

        _qkv_proj_T(nc, ps1_pool, W1, bq1, xT, QT1, 0)
        _qkv_proj_T(nc, ps1_pool, W1, bk1, xT, KT1, DIM)
        # V natural layout [j, c]
        for j in range(JC):
            ps = ps1_pool.tile([128, 512], F32, tag="proj_ps")
            for kc in range(KC):
                nc.tensor.matmul(
                    ps[:],
                    xT[kc][:, j * 128 : (j + 1) * 128],
                    W1[kc][:, 2 * DIM : 3 * DIM],
                    start=(kc == 0),
                    stop=(kc == KC - 1),
                )
            nc.vector.tensor_tensor(V1[j][:], ps[:], bv1_b[:], ALU.add)

    # =====================================================================
    # Phase 2: stage-1 attention (sigmoid(S * mask) @ V), transposed
    # =====================================================================
    with tc.tile_pool(name="p1", bufs=12) as p_pool, \
         tc.tile_pool(name="ptmp", bufs=6) as ptmp_pool, \
         tc.tile_pool(name="sps1", bufs=4, space="PSUM") as score_ps, \
         tc.tile_pool(name="aps1", bufs=4, space="PSUM") as apply_ps:
        for t in range(4):  # head pairs
            P = {}
            for h in (2 * t, 2 * t + 1):
                base = 64 * (h % 2)
                P[h] = [
                    p_pool.tile([128, N], F32R, tag="p", name=f"P1_{h}_{j}")
                    for j in range(JC)
                ]
                for j in range(JC):
                    for ic in range(IC):
                        ps = score_ps.tile([128, 512], F32, tag="score_ps")
                        nc.tensor.matmul(
                            ps[:],
                            KT1[t][base : base + 64, j * 128 : (j + 1) * 128],
                            QT1[t][base : base + 64, ic * 512 : (ic + 1) * 512],
                            start=True,
                            stop=True,
                        )
                        pt = ptmp_pool.tile([128, 512], F32, tag="ptmp")
                        nc.vector.tensor_tensor(
                            pt[:], ps[:], maskT[j][:, ic * 512 : (ic + 1) * 512],
                            ALU.mult,
                        )
                        nc.scalar.activation(
                            P[h][j][:, ic * 512 : (ic + 1) * 512], pt[:], AF.Sigmoid
                        )
            # apply: O1T_h[d, i] += V1_h[j].T @ P_h[j, i]
            aps = {}
            for h in (2 * t, 2 * t + 1):
                aps[h] = [
                    apply_ps.tile([128, 512], F32, tag="apply_ps") for _ in range(IC)
                ]
            for j in range(JC):
                for ic in range(IC):
                    for h in (2 * t, 2 * t + 1):
                        base = 64 * (h % 2)
                        nc.tensor.matmul(
                            aps[h][ic][base : base + 64, :],
                            V1[j][:, h * D : (h + 1) * D],
                            P[h][j][:, ic * 512 : (ic + 1) * 512],
                            start=(j == 0),
                            stop=(j == JC - 1),
                        )
            for h in (2 * t, 2 * t + 1):
                base = 64 * (h % 2)
                for ic in range(IC):
                    nc.vector.tensor_copy(
                        O1T[t][base : base + 64, ic * 512 : (ic + 1) * 512],
                        aps[h][ic][base : base + 64, :],
                    )

    # =====================================================================
    # Phase 3: stage-2 projections (from O1T)
    # =====================================================================
    qk2_pool = ctx.enter_context(tc.tile_pool(name="qk2", bufs=1))
    QT2 = [qk2_pool.tile([128, N], F32R, name=f"QT2_{t}") for t in range(4)]
    KT2 = [qk2_pool.tile([128, N], F32R, name=f"KT2_{t}") for t in range(4)]
    V2p = [qk2_pool.tile([128, H * VP], F32R, name=f"V2p_{j}") for j in range(JC)]

    with tc.tile_pool(name="w2", bufs=1) as w2_pool, \
         tc.tile_pool(name="ps2", bufs=4, space="PSUM") as ps2_pool:
        W2 = [w2_pool.tile([128, 3 * DIM], F32R, name=f"W2_{k}") for k in range(KC)]
        for k in range(KC):
            nc.gpsimd.dma_start(W2[k][:], d["Wqkv2"][k * 128 : (k + 1) * 128, :])

        _qkv_proj_T(nc, ps2_pool, W2, bq2, O1T, QT2, 0)
        _qkv_proj_T(nc, ps2_pool, W2, bk2, O1T, KT2, DIM)
        for j in range(JC):
            ps = ps2_pool.tile([128, 512], F32, tag="proj_ps")
            for kc in range(KC):
                nc.tensor.matmul(
                    ps[:],
                    O1T[kc][:, j * 128 : (j + 1) * 128],
                    W2[kc][:, 2 * DIM : 3 * DIM],
                    start=(kc == 0),
                    stop=(kc == KC - 1),
                )
            # scatter per-head into the padded layout [j, h*65 + d]
            nc.vector.tensor_tensor(
                V2p[j][:, : H * VP].rearrange("p (h e) -> p h e", e=VP)[:, :, :D],
                ps[:].rearrange("p (h dd) -> p h dd", dd=D),
                bv2_b[:].rearrange("p (h dd) -> p h dd", dd=D),
                ALU.add,
            )
            # ones column per head
            nc.gpsimd.dma_start(
                V2p[j][:, : H * VP].rearrange("p (h e) -> p h e", e=VP)[:, :, D:VP],
                d["ones"][:, :],
            )

    # =====================================================================
    # Phase 4: stage-2 attention (softmax via exp + ones-column denominators)
    # =====================================================================
    o2_pool = ctx.enter_context(tc.tile_pool(name="o2", bufs=1))
    O2T = [o2_pool.tile([128, N], F32R, name=f"O2T_{t}") for t in range(4)]

    with tc.tile_pool(name="p2", bufs=12) as p2_pool, \
         tc.tile_pool(name="dscr", bufs=4) as d_pool, \
         tc.tile_pool(name="sps2", bufs=4, space="PSUM") as score2_ps, \
         tc.tile_pool(name="aps2", bufs=2, space="PSUM") as apply2_ps:
        for t in range(4):
            for h in (2 * t, 2 * t + 1):
                base = 64 * (h % 2)
                P2 = [
                    p2_pool.tile([128, N], F32R, tag="p2", name=f"P2_{h}_{j}")
                    for j in range(JC)
                ]
                for j in range(JC):
                    for ic in range(IC):
                        ps = score2_ps.tile([128, 512], F32, tag="score2_ps")
                        nc.tensor.matmul(
                            ps[:],
                            KT2[t][base : base + 64, j * 128 : (j + 1) * 128],
                            QT2[t][base : base + 64, ic * 512 : (ic + 1) * 512],
                            start=True,
                            stop=True,
                        )
                        nc.scalar.activation(
                            P2[j][:, ic * 512 : (ic + 1) * 512],
                            ps[:],
                            AF.Exp,
                            bias=EXP_BIAS,
                            scale=SCALE,
                        )
                # apply with ones column: rows 0:64 = unnormalized out,
                # row 64 = softmax denominator
                aps = [apply2_ps.tile([128, 512], F32, tag="apply2_ps") for _ in range(IC)]
                for j in range(JC):
                    for ic in range(IC):
                        nc.tensor.matmul(
                            aps[ic][: VP, :],
                            V2p[j][:, h * VP : (h + 1) * VP],
                            P2[j][:, ic * 512 : (ic + 1) * 512],
                            start=(j == 0),
                            stop=(j == JC - 1),
                        )
                for ic in range(IC):
                    dsb = d_pool.tile([128, 512], F32, tag="dsb", padded_shape=[128, 512])
                    nc.vector.reciprocal(dsb[64:65, :], aps[ic][64:65, :])
                    db = d_pool.tile([64, 512], F32, tag="db")
                    nc.gpsimd.partition_broadcast(db[:], dsb[64:65, :])
                    if h % 2 == 0:
                        nc.vector.tensor_tensor(
                            O2T[t][0:64, ic * 512 : (ic + 1) * 512],
                            aps[ic][0:64, :],
                            db[:],
                            ALU.mult,
                        )
                    else:
                        ut = d_pool.tile([64, 512], F32R, tag="ut")
                        nc.vector.tensor_tensor(ut[:], aps[ic][0:64, :], db[:], ALU.mult)
                        nc.gpsimd.dma_start(
                            O2T[t][64:128, ic * 512 : (ic + 1) * 512], ut[:]
                        )

    # =====================================================================
    # Phase 5: output projection
    # =====================================================================
    with tc.tile_pool(name="wnn", bufs=1) as wnn_pool, \
         tc.tile_pool(name="outst", bufs=3) as out_pool, \
         tc.tile_pool(name="ps5", bufs=2, space="PSUM") as ps5_pool:
        Wnn = [wnn_pool.tile([128, DIM], F32R, name=f"Wnn_{k}") for k in range(KC)]
        for k in range(KC):
            nc.gpsimd.dma_start(Wnn[k][:], d["Wnn1"][k * 128 : (k + 1) * 128, :])
        for i8 in range(JC):  # 8 chunks of 128 output rows
            ps = ps5_pool.tile([128, 512], F32, tag="out_ps")
            for kc in range(KC):
                nc.tensor.matmul(
                    ps[:],
                    O2T[kc][:, i8 * 128 : (i8 + 1) * 128],
                    Wnn[kc][:],
                    start=(kc == 0),
                    stop=(kc == KC - 1),
                )
            ob = out_pool.tile([128, DIM], F32, tag="ob")
            nc.vector.tensor_tensor(ob[:], ps[:], bnn_b[:], ALU.add)
            nc.gpsimd.dma_start(out_ap[i8 * 128 : (i8 + 1) * 128, :], ob[:])


def build(n_repeat: int = 1):
    install_patches()
    nc = bass.Bass("TRN2", target_bir_lowering=False, debug=False)
    d = {}

    def din(name, shape):
        d[name] = nc.dram_tensor(name, shape, F32, kind="ExternalInput").ap()

    din("xT", [DIM, N])
    din("maskT", [N, N])
    din("Wqkv1", [DIM, 3 * DIM])
    din("bqkv1", [3 * DIM])
    din("Wqkv2", [DIM, 3 * DIM])
    din("bqkv2", [3 * DIM])
    din("Wnn1", [DIM, DIM])
    din("bnn1", [DIM])
    din("ones", [128, H])
    out_ap = nc.dram_tensor("out", [N, DIM], F32, kind="ExternalOutput").ap()

    with tile.TileContext(nc) as tc:
        for _ in range(n_repeat):
            with ExitStack() as ctx:
                build_body(ctx, tc, d, out_ap)

    n = legalize_single_wait(nc)
    return nc, n


# ===========================================================================
# Host-side entry point: full inputs in, full output out.
# Sharding: pure data-parallel — B=8 batch elements, one per NeuronCore.
# ===========================================================================
import numpy as np

_CACHED = {}


def _get_program():
    if "nc" not in _CACHED:
        _CACHED["nc"] = build(n_repeat=1)[0]
    return _CACHED["nc"]


def kernel(x, mask, Wqkv1, bqkv1, Wqkv2, bqkv2, Wnn1, bnn1):
    from concourse.bass_utils import run_bass_kernel_spmd

    x = np.asarray(x, dtype=np.float32)
    maskT = np.ascontiguousarray(np.asarray(mask, dtype=np.float32)[0, 0].T)
    common = {
        "maskT": maskT,
        "Wqkv1": np.ascontiguousarray(np.asarray(Wqkv1, dtype=np.float32)),
        "bqkv1": np.ascontiguousarray(np.asarray(bqkv1, dtype=np.float32)),
        "Wqkv2": np.ascontiguousarray(np.asarray(Wqkv2, dtype=np.float32)),
        "bqkv2": np.ascontiguousarray(np.asarray(bqkv2, dtype=np.float32)),
        "Wnn1": np.ascontiguousarray(np.asarray(Wnn1, dtype=np.float32)),
        "bnn1": np.ascontiguousarray(np.asarray(bnn1, dtype=np.float32)),
        "ones": np.ones((128, H), dtype=np.float32),
    }
    in_maps = [
        {"xT": np.ascontiguousarray(x[c].T), **common} for c in range(x.shape[0])
    ]
    nc = _get_program()
    res = run_bass_kernel_spmd(nc, in_maps, core_ids=list(range(8)))
    return np.stack([res.results[c]["out"] for c in range(8)]).astype(np.float32)
